# revision 32
# baseline (speedup 1.0000x reference)
"""Trainium2 Bass kernel for segment-mean -> gated-MLP -> gather-gate (nn_Context).

Math (reference):
    seg_sum[s] = sum_{n: bid[n]==s} h_V[n]          # [S, H]
    c_V = seg_sum / max(counts, 1)                  # [S, H]
    hdn = relu(c_V @ W1.T + b1)
    gate = sigmoid(hdn @ W2.T + b2)                 # [S, H]
    out[n] = h_V[n] * gate[bid[n]]                  # [N, H]

Strategy: shard nodes equally across 8 cores; h_V staged to DRAM in bf16
(tolerance 2e-2 >> bf16 rounding ~4e-3). Each core:
  pass 1: stream its h_V slice in 1 MiB tiles (8 blocks of 128 nodes per
          DMA); per block build a one-hot [128, S] on DVE and accumulate
          seg_sum [S, H] / counts [S, 1] on the PE into PSUM.  The tail
          CI iterations stay resident in SBUF (cache) for pass 2.
  AllReduce the packed [S, H+1] partials across the 8 cores.
  Tiny replicated MLP (bf16 weights) on PE + ACT -> gate [S, H] bf16.
  pass 2: per iteration build the transposed one-hot [S, L] (gpsimd
          partition-broadcast of the bid row + DVE is_equal), PE-matmul
          gathers gate rows, multiply with h_V (cached or re-read) and
          DMA out in bf16.  Host upcasts the result to f32.
"""

import numpy as np

import concourse.bass as bass
import concourse.mybir as mybir
import concourse.tile as tile
from concourse import bacc
from concourse.bass_utils import run_bass_kernel_spmd

N = 262144
H = 512
S = 64
CORES = 8
NPC = N // CORES       # 32768 nodes per core
BLK = 128              # nodes per matmul block (partition dim)
BPI = 8                # blocks per DMA iteration
L = BLK * BPI          # 1024 nodes per iteration
ITERS = NPC // L       # 32 iterations per pass
NBLK = NPC // BLK      # 256 matmul blocks
CI = 13                # iterations cached in SBUF between passes
CUT = ITERS - CI       # first cached iteration
KC = H // 128          # 4 column chunks of the hidden dim
F32 = mybir.dt.float32
BF16 = mybir.dt.bfloat16
F8 = mybir.dt.float8e4

EQ = mybir.AluOpType.is_equal
MULT = mybir.AluOpType.mult
AF = mybir.ActivationFunctionType

_cached = None  # (nc,) built once per process


def _is_cached(j):
    # Cached iterations are interleaved with streaming ones (odd j) so the
    # DMA engines never sit idle through a long cached-only tail: a streaming
    # iteration's input DMA overlaps its cached neighbour's compute.
    return j % 2 == 1 and j < 2 * CI


def _act_pairs(j):
    # Gating-multiply engine split per iteration (4 block-pairs each): pairs
    # below the returned count copy the gathered gate rows PSUM->SBUF (bf16,
    # ACT) so the DVE multiply runs in fast 16-bit mode; the rest multiply
    # straight from PSUM (f32) on DVE.  Cached iterations have no input DMA,
    # so ACT would otherwise become their pacer -> give one pair back to DVE.
    return 3 if _is_cached(j) else 4


def _build(use_collective=True, reps=1):
    nc = bacc.Bacc("TRN2", target_bir_lowering=False, debug=False,
                   num_devices=CORES if use_collective else None)

    hv_d = nc.dram_tensor("hv", [NPC, H], BF16, kind="ExternalInput")
    hv8_d = nc.dram_tensor("hv8", [CUT * L, H], F8, kind="ExternalInput")
    bidq_d = nc.dram_tensor("bidq", [BLK, NBLK], F32, kind="ExternalInput")
    bidr_d = nc.dram_tensor("bidr", [1, NPC], BF16, kind="ExternalInput")
    w1t_d = nc.dram_tensor("w1t", [128, KC, H], BF16, kind="ExternalInput")
    w2t_d = nc.dram_tensor("w2t", [128, KC, H], BF16, kind="ExternalInput")
    b1c_d = nc.dram_tensor("b1c", [128, KC], F32, kind="ExternalInput")
    b2c_d = nc.dram_tensor("b2c", [128, KC], F32, kind="ExternalInput")
    iotar_d = nc.dram_tensor("iotar", [128, S], BF16, kind="ExternalInput")
    iotac_d = nc.dram_tensor("iotac", [S, 1], F32, kind="ExternalInput")
    ones_d = nc.dram_tensor("ones", [BLK, 1], BF16, kind="ExternalInput")
    ones8_d = nc.dram_tensor("ones8", [BLK, 1], F8, kind="ExternalInput")
    ident_d = nc.dram_tensor("ident", [128, 128], BF16, kind="ExternalInput")
    out_d = nc.dram_tensor("out", [NPC, H], BF16, kind="ExternalOutput")

    # [i, p, b, h]: iteration i, block b, node = i*L + p*BPI + b
    hv_v = hv_d.ap().rearrange("(i p b) h -> i p b h", p=BLK, b=BPI)
    hv8_v = hv8_d.ap().rearrange("(i p b) h -> i p b h", p=BLK, b=BPI)
    out_v = out_d.ap().rearrange("(i p b) h -> i p b h", p=BLK, b=BPI)
    bidr_ap = bidr_d.ap()

    with tile.TileContext(nc) as tc:
        with (
            tc.tile_pool(name="const", bufs=1) as constp,
            tc.tile_pool(name="cache", bufs=1) as cachep,
            tc.tile_pool(name="hvp", bufs=4) as hvp,
            tc.tile_pool(name="hv8p", bufs=3) as hv8p,
            tc.tile_pool(name="rowp", bufs=2) as rowp,
            tc.tile_pool(name="ohp", bufs=4) as ohp,
            tc.tile_pool(name="bcp", bufs=3) as bcp,
            tc.tile_pool(name="ohtp", bufs=3) as ohtp,
            tc.tile_pool(name="gsbp", bufs=4) as gsbp,
            tc.tile_pool(name="outp", bufs=2) as outp,
            tc.tile_pool(name="smallp", bufs=1) as smallp,
            tc.tile_pool(name="dramp", bufs=1, space="DRAM") as dramp,
        ):
            # ---- constants ----
            bidq = constp.tile([BLK, NBLK], F32, tag="bidq")
            nc.scalar.dma_start(out=bidq[:], in_=bidq_d.ap())
            iotar = constp.tile([128, S], BF16, tag="iotar")
            nc.scalar.dma_start(out=iotar[:], in_=iotar_d.ap())
            iotac = constp.tile([S, 1], F32, tag="iotac")
            nc.scalar.dma_start(out=iotac[:], in_=iotac_d.ap())
            ones = constp.tile([BLK, 1], BF16, tag="ones")
            nc.scalar.dma_start(out=ones[:], in_=ones_d.ap())
            ones8 = constp.tile([BLK, 1], F8, tag="ones8")
            nc.scalar.dma_start(out=ones8[:], in_=ones8_d.ap())
            ident = constp.tile([128, 128], BF16, tag="ident")
            nc.scalar.dma_start(out=ident[:], in_=ident_d.ap())
            w1t = constp.tile([128, KC, H], BF16, tag="w1t")
            nc.scalar.dma_start(out=w1t[:], in_=w1t_d.ap())
            w2t = constp.tile([128, KC, H], BF16, tag="w2t")
            nc.scalar.dma_start(out=w2t[:], in_=w2t_d.ap())
            b1c = constp.tile([128, KC], F32, tag="b1c")
            nc.scalar.dma_start(out=b1c[:], in_=b1c_d.ap())
            b2c = constp.tile([128, KC], F32, tag="b2c")
            nc.scalar.dma_start(out=b2c[:], in_=b2c_d.ap())
            gate = constp.tile([S, H], BF16, tag="gate")
            hvc = cachep.tile([128, CI, BPI, H], BF16, tag="hvc")

            def body():
                _body(nc, tc, hv_v, hv8_v, out_v, bidr_ap, bidq, iotar, iotac,
                      ones, ones8, ident, w1t, w2t, b1c, b2c, gate, hvc, hvp,
                      hv8p, rowp, ohp, bcp, ohtp, gsbp, outp, smallp, dramp,
                      use_collective)

            if reps == 1:
                body()
            else:
                with tc.For_i(0, reps, 1):
                    body()

    nc.compile()
    return nc


def _body(nc, tc, hv_v, hv8_v, out_v, bidr_ap, bidq, iotar, iotac, ones,
          ones8, ident, w1t, w2t, b1c, b2c, gate, hvc, hvp, hv8p, rowp, ohp,
          bcp, ohtp, gsbp, outp, smallp, dramp, use_collective):
    with tc.tile_pool(name="psacc", bufs=1, space="PSUM") as psacc, \
         tc.tile_pool(name="psmlp", bufs=3, space="PSUM") as psmlp:
        # ---- pass 1: per-core seg_sum [S, H] and counts [S, 1] ----
        # Uncached iterations read an fp8 staging of h_V: only the segment
        # MEANS flow through it (error averages down over ~4k nodes/segment
        # and the gate is flat around 0.5), the gating multiply in pass 2
        # always uses bf16 data.
        pseg = psacc.tile([S, H], F32, tag="pseg")
        pcnt = psacc.tile([S, 1], F32, tag="pcnt")

        u = 0  # uncached-iteration ordinal (indexes the fp8 staging)
        for j in range(ITERS):
            f8 = not _is_cached(j)
            if not f8:
                hv_t = hvc[:, j // 2]
                nc.sync.dma_start(out=hv_t, in_=hv_v[j])
            else:
                hv_new = hv8p.tile([BLK, BPI, H], F8, tag="hv8")
                hv_t = hv_new[:]
                nc.sync.dma_start(out=hv_t, in_=hv8_v[u])
                u += 1
            for b in range(BPI):
                i = j * BPI + b
                oh = ohp.tile([BLK, S], F8 if f8 else BF16,
                              tag="oh8" if f8 else "oh")
                nc.vector.tensor_scalar(
                    out=oh[:], in0=iotar[:],
                    scalar1=bidq[:, i:i + 1], scalar2=None, op0=EQ)
                first = i == 0
                last = i == NBLK - 1
                nc.tensor.matmul(pseg[:], lhsT=oh[:], rhs=hv_t[:, b, :],
                                 start=first, stop=last)
                nc.tensor.matmul(pcnt[:], lhsT=oh[:],
                                 rhs=ones8[:] if f8 else ones[:],
                                 start=first, stop=last)

        # ---- AllReduce partial stats across the 8 cores ----
        pack = smallp.tile([S, H + 1], F32, tag="pack")
        nc.scalar.copy(pack[:, :H], pseg[:])
        nc.scalar.copy(pack[:, H:H + 1], pcnt[:])
        cc_in = dramp.tile([S, H + 1], F32, tag="ccin")
        cc_out = dramp.tile([S, H + 1], F32, tag="ccout")
        # Collective staging DMAs stay off the SP queue (pass-2 loads would
        # park behind the AllReduce wait) and use HWDGE queues (Act) rather
        # than Pool's SWDGE, whose fixed overheads sit on the serial mid
        # chain.
        nc.scalar.dma_start(out=cc_in[:], in_=pack[:])
        if use_collective:
            nc.gpsimd.collective_compute(
                "AllReduce",
                mybir.AluOpType.add,
                replica_groups=[list(range(CORES))],
                ins=[cc_in[:].opt()],
                outs=[cc_out[:].opt()],
            )
        else:  # single-core timing-model variant
            nc.gpsimd.dma_start(out=cc_out[:], in_=cc_in[:])
        packr = smallp.tile([S, H + 1], F32, tag="packr")
        nc.scalar.dma_start(out=packr[:], in_=cc_out[:])

        # ---- c_V = seg_sum / max(counts, 1) ----
        cnt = smallp.tile([S, 1], F32, tag="cnt")
        nc.vector.tensor_scalar_max(cnt[:], packr[:, H:H + 1], 1.0)
        rcp = smallp.tile([S, 1], F32, tag="rcp")
        nc.vector.reciprocal(rcp[:], cnt[:])
        cv = smallp.tile([S, H], BF16, tag="cv")
        nc.vector.tensor_scalar_mul(cv[:], packr[:, :H], rcp[:])

        # ---- transpose c_V -> ct [128, kc, S] (k on partitions) ----
        ct = smallp.tile([128, KC, S], BF16, tag="ct")
        for kc in range(KC):
            pt = psmlp.tile([128, S], BF16, tag="mlpt")
            nc.tensor.transpose(pt[:], in_=cv[:, kc * 128:(kc + 1) * 128],
                                identity=ident[:S, :S])
            nc.scalar.copy(ct[:, kc, :], pt[:])

        # ---- layer 1: h1_T[j, s] = relu(W1 @ c_V.T + b1) ----
        h1 = smallp.tile([128, KC, S], BF16, tag="h1")
        for jc in range(KC):
            ph = psmlp.tile([128, S], F32, tag="mlp")
            for kc in range(KC):
                nc.tensor.matmul(
                    ph[:], lhsT=w1t[:, kc, jc * 128:(jc + 1) * 128],
                    rhs=ct[:, kc, :], start=kc == 0, stop=kc == KC - 1)
            nc.scalar.activation(h1[:, jc, :], ph[:], AF.Relu,
                                 bias=b1c[:, jc:jc + 1])

        # ---- layer 2: g_T[m, s] = sigmoid(W2 @ h1 + b2) ----
        gt = smallp.tile([128, KC, S], BF16, tag="gt")
        for mc in range(KC):
            ph = psmlp.tile([128, S], F32, tag="mlp")
            for jc in range(KC):
                nc.tensor.matmul(
                    ph[:], lhsT=w2t[:, jc, mc * 128:(mc + 1) * 128],
                    rhs=h1[:, jc, :], start=jc == 0, stop=jc == KC - 1)
            nc.scalar.activation(gt[:, mc, :], ph[:], AF.Sigmoid,
                                 bias=b2c[:, mc:mc + 1])

        # ---- transpose back: gate [S, H] bf16 ----
        for mc in range(KC):
            pg = psmlp.tile([S, 128], BF16, tag="mlpt")
            nc.tensor.transpose(pg[:], in_=gt[:, mc, :], identity=ident[:])
            nc.scalar.copy(gate[:, mc * 128:(mc + 1) * 128], pg[:])

    # ---- pass 2: out = h_V * gate[bid] ----
    with tc.tile_pool(name="psg", bufs=4, space="PSUM") as psg:
        for j in range(ITERS):
            rowt = rowp.tile([1, L], BF16, tag="row")
            nc.sync.dma_start(out=rowt[:], in_=bidr_ap[0:1, j * L:(j + 1) * L])
            bct = bcp.tile([S, L], BF16, tag="bc")
            nc.gpsimd.partition_broadcast(bct[:], rowt[0:1, :], channels=S)
            oht = ohtp.tile([S, L], BF16, tag="oht")
            nc.vector.tensor_scalar(
                out=oht[:], in0=bct[:],
                scalar1=iotac[:], scalar2=None, op0=EQ)
            if _is_cached(j):
                hv_t = hvc[:, j // 2]
            else:
                hv_new = hvp.tile([BLK, BPI, H], BF16, tag="hv1")
                hv_t = hv_new[:]
                nc.sync.dma_start(out=hv_t, in_=hv_v[j])
            ot = outp.tile([BLK, BPI, H], BF16, tag="ot")
            for q in range(BPI // 2):
                pgt = psg.tile([BLK, 2, H], F32, tag="pg2")
                for r in range(2):
                    b = 2 * q + r
                    nc.tensor.matmul(
                        pgt[:, r, :],
                        lhsT=oht[:, b * BLK:(b + 1) * BLK],
                        rhs=gate[:], start=True, stop=True)
                if q < _act_pairs(j):
                    gsb = gsbp.tile([BLK, 2, H], BF16, tag="gsb")
                    nc.scalar.copy(gsb[:], pgt[:])
                    nc.vector.tensor_tensor(
                        out=ot[:, 2 * q:2 * q + 2, :],
                        in0=hv_t[:, 2 * q:2 * q + 2, :], in1=gsb[:], op=MULT)
                else:
                    nc.vector.tensor_tensor(
                        out=ot[:, 2 * q:2 * q + 2, :],
                        in0=hv_t[:, 2 * q:2 * q + 2, :], in1=pgt[:], op=MULT)
            # out-DMA on SP: its wait (multiply done) matches the WAR waits
            # the SP prefetch loads already park on, so it adds no new
            # serialization; on Pool it blocked partition_broadcast, on Act
            # it stalled the PSUM->SBUF copy chain.  Two half-iteration DMAs
            # let the write start after half the multiplies.
            nc.sync.dma_start(out=out_v[j][:, 0:BPI // 2, :],
                              in_=ot[:, 0:BPI // 2, :])
            nc.sync.dma_start(out=out_v[j][:, BPI // 2:BPI, :],
                              in_=ot[:, BPI // 2:BPI, :])


def _prep_inputs(inputs):
    import ml_dtypes
    bf16 = ml_dtypes.bfloat16
    f8 = ml_dtypes.float8_e4m3

    h_V = np.asarray(inputs["h_V"], dtype=np.float32)
    bid = np.asarray(inputs["batch_id"])
    W1 = np.asarray(inputs["W1"], dtype=np.float32)
    b1 = np.asarray(inputs["b1"], dtype=np.float32)
    W2 = np.asarray(inputs["W2"], dtype=np.float32)
    b2 = np.asarray(inputs["b2"], dtype=np.float32)

    h_bf = h_V.astype(bf16)
    bid_f = bid.astype(np.float32)
    w1t = np.ascontiguousarray(
        W1.T.reshape(KC, 128, H).transpose(1, 0, 2)).astype(bf16)
    w2t = np.ascontiguousarray(
        W2.T.reshape(KC, 128, H).transpose(1, 0, 2)).astype(bf16)
    b1c = np.ascontiguousarray(b1.reshape(KC, 128).T)
    b2c = np.ascontiguousarray(b2.reshape(KC, 128).T)
    iotar = np.ascontiguousarray(
        np.tile(np.arange(S, dtype=bf16), (128, 1)))
    iotac = np.arange(S, dtype=np.float32).reshape(S, 1)
    ones = np.ones((BLK, 1), dtype=bf16)
    ones8 = np.ones((BLK, 1), dtype=f8)
    ident = np.eye(128, dtype=bf16)

    in_maps = []
    for c in range(CORES):
        lo, hi = c * NPC, (c + 1) * NPC
        bid_c = bid_f[lo:hi]
        # bidq[p, j*BPI+b] = bid[j*L + p*BPI + b] (pass-1 per-block scalars)
        bidq = np.ascontiguousarray(
            bid_c.reshape(ITERS, BLK, BPI).transpose(1, 0, 2)
            .reshape(BLK, NBLK))
        # bidr[j*L + b*BLK + p] = bid[j*L + p*BPI + b] (pass-2 one-hot is
        # block-major so oht columns for block b are contiguous)
        bidr = np.ascontiguousarray(
            bid_c.reshape(ITERS, BLK, BPI).transpose(0, 2, 1)
            .reshape(1, NPC)).astype(bf16)
        unc = [j for j in range(ITERS) if not _is_cached(j)]
        in_maps.append({
            "hv": h_bf[lo:hi],
            "hv8": np.concatenate(
                [h_V[lo + j * L:lo + (j + 1) * L] for j in unc]).astype(f8),
            "bidq": bidq,
            "bidr": bidr,
            "w1t": w1t, "w2t": w2t, "b1c": b1c, "b2c": b2c,
            "iotar": iotar, "iotac": iotac, "ones": ones, "ones8": ones8,
            "ident": ident,
        })
    return in_maps


def _run(inputs, trace=False):
    global _cached
    if _cached is None:
        _cached = _build()
    nc = _cached
    in_maps = _prep_inputs(inputs)
    res = run_bass_kernel_spmd(nc, in_maps, core_ids=list(range(CORES)),
                               trace=trace)
    out = np.concatenate(
        [np.asarray(res.results[c]["out"]).astype(np.float32)
         for c in range(CORES)], axis=0)
    return out, res


def kernel(**inputs) -> np.ndarray:
    out, _ = _run(inputs, trace=False)
    return out


# revision 33
# speedup vs baseline: 1.1241x; 1.1241x over previous
"""Trainium2 Bass kernel for segment-mean -> gated-MLP -> gather-gate (nn_Context).

Math (reference):
    seg_sum[s] = sum_{n: bid[n]==s} h_V[n]          # [S, H]
    c_V = seg_sum / max(counts, 1)                  # [S, H]
    hdn = relu(c_V @ W1.T + b1)
    gate = sigmoid(hdn @ W2.T + b2)                 # [S, H]
    out[n] = h_V[n] * gate[bid[n]]                  # [N, H]

Strategy: shard nodes equally across 8 cores; h_V staged to DRAM in bf16
(tolerance 2e-2 >> bf16 rounding ~4e-3). Each core:
  pass 1: stream its h_V slice in 1 MiB tiles (8 blocks of 128 nodes per
          DMA); per block build a one-hot [128, S] on DVE and accumulate
          seg_sum [S, H] / counts [S, 1] on the PE into PSUM.  The tail
          CI iterations stay resident in SBUF (cache) for pass 2.
  AllReduce the packed [S, H+1] partials across the 8 cores.
  Tiny replicated MLP (bf16 weights) on PE + ACT -> gate [S, H] bf16.
  pass 2: per iteration build the transposed one-hot [S, L] (gpsimd
          partition-broadcast of the bid row + DVE is_equal), PE-matmul
          gathers gate rows, multiply with h_V (cached or re-read) and
          DMA out in bf16.  Host upcasts the result to f32.
"""

import numpy as np

import concourse.bass as bass
import concourse.mybir as mybir
import concourse.tile as tile
from concourse import bacc
from concourse.bass_utils import run_bass_kernel_spmd

N = 262144
H = 512
S = 64
CORES = 8
NPC = N // CORES       # 32768 nodes per core
BLK = 128              # nodes per matmul block (partition dim)
BPI = 8                # blocks per DMA iteration
L = BLK * BPI          # 1024 nodes per iteration
ITERS = NPC // L       # 32 iterations per pass
NBLK = NPC // BLK      # 256 matmul blocks
CI = 13                # iterations cached in SBUF between passes
CUT = ITERS - CI       # first cached iteration
KC = H // 128          # 4 column chunks of the hidden dim
F32 = mybir.dt.float32
BF16 = mybir.dt.bfloat16
F8 = mybir.dt.float8e4

EQ = mybir.AluOpType.is_equal
MULT = mybir.AluOpType.mult
AF = mybir.ActivationFunctionType

_cached = None  # (nc,) built once per process


def _is_cached(j):
    # Cached iterations are interleaved with streaming ones (odd j) so the
    # DMA engines never sit idle through a long cached-only tail: a streaming
    # iteration's input DMA overlaps its cached neighbour's compute.
    return j % 2 == 1 and j < 2 * CI


def _act_pairs(j):
    # Gating-multiply engine split per iteration (4 block-pairs each): pairs
    # below the returned count copy the gathered gate rows PSUM->SBUF (bf16,
    # ACT) so the DVE multiply runs in fast 16-bit mode; the rest multiply
    # straight from PSUM (f32) on DVE.  Cached iterations have no input DMA,
    # so ACT would otherwise become their pacer -> give one pair back to DVE.
    return 3 if _is_cached(j) else 4


def _build(use_collective=True, reps=1):
    nc = bacc.Bacc("TRN2", target_bir_lowering=False, debug=False,
                   num_devices=CORES if use_collective else None)

    hv_d = nc.dram_tensor("hv", [NPC, H], BF16, kind="ExternalInput")
    hv8_d = nc.dram_tensor("hv8", [CUT * L, H], F8, kind="ExternalInput")
    bidq_d = nc.dram_tensor("bidq", [BLK, NBLK], F32, kind="ExternalInput")
    bidr_d = nc.dram_tensor("bidr", [1, NPC], BF16, kind="ExternalInput")
    w1t_d = nc.dram_tensor("w1t", [128, KC, H], BF16, kind="ExternalInput")
    w2t_d = nc.dram_tensor("w2t", [128, KC, H], BF16, kind="ExternalInput")
    b1c_d = nc.dram_tensor("b1c", [128, KC], F32, kind="ExternalInput")
    b2c_d = nc.dram_tensor("b2c", [128, KC], F32, kind="ExternalInput")
    iotar_d = nc.dram_tensor("iotar", [128, S], BF16, kind="ExternalInput")
    iotac_d = nc.dram_tensor("iotac", [S, 1], F32, kind="ExternalInput")
    ones_d = nc.dram_tensor("ones", [BLK, 1], BF16, kind="ExternalInput")
    ones8_d = nc.dram_tensor("ones8", [BLK, 1], F8, kind="ExternalInput")
    ident_d = nc.dram_tensor("ident", [128, 128], BF16, kind="ExternalInput")
    out_d = nc.dram_tensor("out", [NPC, H], BF16, kind="ExternalOutput")

    # [i, p, b, h]: iteration i, block b, node = i*L + p*BPI + b
    hv_v = hv_d.ap().rearrange("(i p b) h -> i p b h", p=BLK, b=BPI)
    hv8_v = hv8_d.ap().rearrange("(i p b) h -> i p b h", p=BLK, b=BPI)
    out_v = out_d.ap().rearrange("(i p b) h -> i p b h", p=BLK, b=BPI)
    bidr_ap = bidr_d.ap()

    with tile.TileContext(nc) as tc:
        with (
            tc.tile_pool(name="const", bufs=1) as constp,
            tc.tile_pool(name="cache", bufs=1) as cachep,
            tc.tile_pool(name="hvp", bufs=4) as hvp,
            tc.tile_pool(name="hv8p", bufs=3) as hv8p,
            tc.tile_pool(name="rowp", bufs=2) as rowp,
            tc.tile_pool(name="ohp", bufs=4) as ohp,
            tc.tile_pool(name="bcp", bufs=3) as bcp,
            tc.tile_pool(name="ohtp", bufs=3) as ohtp,
            tc.tile_pool(name="gsbp", bufs=4) as gsbp,
            tc.tile_pool(name="outp", bufs=2) as outp,
            tc.tile_pool(name="smallp", bufs=1) as smallp,
            tc.tile_pool(name="dramp", bufs=1, space="DRAM") as dramp,
        ):
            # ---- constants ----
            bidq = constp.tile([BLK, NBLK], F32, tag="bidq")
            nc.scalar.dma_start(out=bidq[:], in_=bidq_d.ap())
            iotar = constp.tile([128, S], BF16, tag="iotar")
            nc.scalar.dma_start(out=iotar[:], in_=iotar_d.ap())
            iotac = constp.tile([S, 1], F32, tag="iotac")
            nc.scalar.dma_start(out=iotac[:], in_=iotac_d.ap())
            ones = constp.tile([BLK, 1], BF16, tag="ones")
            nc.scalar.dma_start(out=ones[:], in_=ones_d.ap())
            ones8 = constp.tile([BLK, 1], F8, tag="ones8")
            nc.scalar.dma_start(out=ones8[:], in_=ones8_d.ap())
            ident = constp.tile([128, 128], BF16, tag="ident")
            nc.scalar.dma_start(out=ident[:], in_=ident_d.ap())
            w1t = constp.tile([128, KC, H], BF16, tag="w1t")
            nc.scalar.dma_start(out=w1t[:], in_=w1t_d.ap())
            w2t = constp.tile([128, KC, H], BF16, tag="w2t")
            nc.scalar.dma_start(out=w2t[:], in_=w2t_d.ap())
            b1c = constp.tile([128, KC], F32, tag="b1c")
            nc.scalar.dma_start(out=b1c[:], in_=b1c_d.ap())
            b2c = constp.tile([128, KC], F32, tag="b2c")
            nc.scalar.dma_start(out=b2c[:], in_=b2c_d.ap())
            gate = constp.tile([S, H], BF16, tag="gate")
            hvc = cachep.tile([128, CI, BPI, H], BF16, tag="hvc")

            def body():
                _body(nc, tc, hv_v, hv8_v, out_v, bidr_ap, bidq, iotar, iotac,
                      ones, ones8, ident, w1t, w2t, b1c, b2c, gate, hvc, hvp,
                      hv8p, rowp, ohp, bcp, ohtp, gsbp, outp, smallp, dramp,
                      use_collective)

            if reps == 1:
                body()
            else:
                with tc.For_i(0, reps, 1):
                    body()

    nc.compile()
    return nc


def _body(nc, tc, hv_v, hv8_v, out_v, bidr_ap, bidq, iotar, iotac, ones,
          ones8, ident, w1t, w2t, b1c, b2c, gate, hvc, hvp, hv8p, rowp, ohp,
          bcp, ohtp, gsbp, outp, smallp, dramp, use_collective):
    with tc.tile_pool(name="psacc", bufs=1, space="PSUM") as psacc, \
         tc.tile_pool(name="psmlp", bufs=3, space="PSUM") as psmlp:
        # ---- pass 1: per-core seg_sum [S, H] and counts [S, 1] ----
        # Uncached iterations read an fp8 staging of h_V: only the segment
        # MEANS flow through it (error averages down over ~4k nodes/segment
        # and the gate is flat around 0.5), the gating multiply in pass 2
        # always uses bf16 data.
        pseg = psacc.tile([S, H], F32, tag="pseg")
        pcnt = psacc.tile([S, 1], F32, tag="pcnt")

        u = 0  # uncached-iteration ordinal (indexes the fp8 staging)
        for j in range(ITERS):
            f8 = not _is_cached(j)
            if not f8:
                hv_t = hvc[:, j // 2]
                nc.sync.dma_start(out=hv_t, in_=hv_v[j])
            else:
                hv_new = hv8p.tile([BLK, BPI, H], F8, tag="hv8")
                hv_t = hv_new[:]
                nc.sync.dma_start(out=hv_t, in_=hv8_v[u])
                u += 1
            for b in range(BPI):
                i = j * BPI + b
                oh = ohp.tile([BLK, S], F8 if f8 else BF16,
                              tag="oh8" if f8 else "oh")
                nc.vector.tensor_scalar(
                    out=oh[:], in0=iotar[:],
                    scalar1=bidq[:, i:i + 1], scalar2=None, op0=EQ)
                first = i == 0
                last = i == NBLK - 1
                nc.tensor.matmul(pseg[:], lhsT=oh[:], rhs=hv_t[:, b, :],
                                 start=first, stop=last)
                nc.tensor.matmul(pcnt[:], lhsT=oh[:],
                                 rhs=ones8[:] if f8 else ones[:],
                                 start=first, stop=last)

        # ---- AllReduce partial stats across the 8 cores ----
        pack = smallp.tile([S, H + 1], F32, tag="pack")
        nc.scalar.copy(pack[:, :H], pseg[:])
        nc.scalar.copy(pack[:, H:H + 1], pcnt[:])
        cc_in = dramp.tile([S, H + 1], F32, tag="ccin")
        cc_out = dramp.tile([S, H + 1], F32, tag="ccout")
        # Collective staging DMAs stay off the SP queue (pass-2 loads would
        # park behind the AllReduce wait) and use HWDGE queues (Act) rather
        # than Pool's SWDGE, whose fixed overheads sit on the serial mid
        # chain.
        nc.scalar.dma_start(out=cc_in[:], in_=pack[:])
        if use_collective:
            nc.gpsimd.collective_compute(
                "AllReduce",
                mybir.AluOpType.add,
                replica_groups=[list(range(CORES))],
                ins=[cc_in[:].opt()],
                outs=[cc_out[:].opt()],
            )
        else:  # single-core timing-model variant
            nc.gpsimd.dma_start(out=cc_out[:], in_=cc_in[:])
        packr = smallp.tile([S, H + 1], F32, tag="packr")
        nc.scalar.dma_start(out=packr[:], in_=cc_out[:])

        # ---- c_V = seg_sum / max(counts, 1) ----
        cnt = smallp.tile([S, 1], F32, tag="cnt")
        nc.vector.tensor_scalar_max(cnt[:], packr[:, H:H + 1], 1.0)
        rcp = smallp.tile([S, 1], F32, tag="rcp")
        nc.vector.reciprocal(rcp[:], cnt[:])
        cv = smallp.tile([S, H], BF16, tag="cv")
        nc.vector.tensor_scalar_mul(cv[:], packr[:, :H], rcp[:])

        # ---- transpose c_V -> ct [128, kc, S] (k on partitions) ----
        ct = smallp.tile([128, KC, S], BF16, tag="ct")
        for kc in range(KC):
            pt = psmlp.tile([128, S], BF16, tag="mlpt")
            nc.tensor.transpose(pt[:], in_=cv[:, kc * 128:(kc + 1) * 128],
                                identity=ident[:S, :S])
            nc.scalar.copy(ct[:, kc, :], pt[:])

        # ---- layer 1: h1_T[j, s] = relu(W1 @ c_V.T + b1) ----
        h1 = smallp.tile([128, KC, S], BF16, tag="h1")
        for jc in range(KC):
            ph = psmlp.tile([128, S], F32, tag="mlp")
            for kc in range(KC):
                nc.tensor.matmul(
                    ph[:], lhsT=w1t[:, kc, jc * 128:(jc + 1) * 128],
                    rhs=ct[:, kc, :], start=kc == 0, stop=kc == KC - 1)
            nc.scalar.activation(h1[:, jc, :], ph[:], AF.Relu,
                                 bias=b1c[:, jc:jc + 1])

        # ---- layer 2: g_T[m, s] = sigmoid(W2 @ h1 + b2) ----
        gt = smallp.tile([128, KC, S], BF16, tag="gt")
        for mc in range(KC):
            ph = psmlp.tile([128, S], F32, tag="mlp")
            for jc in range(KC):
                nc.tensor.matmul(
                    ph[:], lhsT=w2t[:, jc, mc * 128:(mc + 1) * 128],
                    rhs=h1[:, jc, :], start=jc == 0, stop=jc == KC - 1)
            nc.scalar.activation(gt[:, mc, :], ph[:], AF.Sigmoid,
                                 bias=b2c[:, mc:mc + 1])

        # ---- transpose back: gate [S, H] bf16 ----
        for mc in range(KC):
            pg = psmlp.tile([S, 128], BF16, tag="mlpt")
            nc.tensor.transpose(pg[:], in_=gt[:, mc, :], identity=ident[:])
            nc.scalar.copy(gate[:, mc * 128:(mc + 1) * 128], pg[:])

    # ---- pass 2: out = h_V * gate[bid] ----
    with tc.tile_pool(name="psg", bufs=4, space="PSUM") as psg:
        for j in range(ITERS):
            rowt = rowp.tile([1, L], BF16, tag="row")
            nc.sync.dma_start(out=rowt[:], in_=bidr_ap[0:1, j * L:(j + 1) * L])
            bct = bcp.tile([S, L], BF16, tag="bc")
            nc.gpsimd.partition_broadcast(bct[:], rowt[0:1, :], channels=S)
            oht = ohtp.tile([S, L], BF16, tag="oht")
            nc.vector.tensor_scalar(
                out=oht[:], in0=bct[:],
                scalar1=iotac[:], scalar2=None, op0=EQ)
            if _is_cached(j):
                hv_t = hvc[:, j // 2]
            else:
                hv_new = hvp.tile([BLK, BPI, H], BF16, tag="hv1")
                hv_t = hv_new[:]
                nc.sync.dma_start(out=hv_t, in_=hv_v[j])
            ot = outp.tile([BLK, BPI, H], BF16, tag="ot")
            for q in range(BPI // 2):
                pgt = psg.tile([BLK, 2, H], F32, tag="pg2")
                for r in range(2):
                    b = 2 * q + r
                    nc.tensor.matmul(
                        pgt[:, r, :],
                        lhsT=oht[:, b * BLK:(b + 1) * BLK],
                        rhs=gate[:], start=True, stop=True)
                if q < _act_pairs(j):
                    gsb = gsbp.tile([BLK, 2, H], BF16, tag="gsb")
                    nc.scalar.copy(gsb[:], pgt[:])
                    nc.vector.tensor_tensor(
                        out=ot[:, 2 * q:2 * q + 2, :],
                        in0=hv_t[:, 2 * q:2 * q + 2, :], in1=gsb[:], op=MULT)
                else:
                    nc.vector.tensor_tensor(
                        out=ot[:, 2 * q:2 * q + 2, :],
                        in0=hv_t[:, 2 * q:2 * q + 2, :], in1=pgt[:], op=MULT)
            # out-DMA on SP: its wait (multiply done) matches the WAR waits
            # the SP prefetch loads already park on, so it adds no new
            # serialization; on Pool it blocked partition_broadcast, on Act
            # it stalled the PSUM->SBUF copy chain.
            nc.sync.dma_start(out=out_v[j], in_=ot[:])


def _prep_inputs(inputs):
    import ml_dtypes
    bf16 = ml_dtypes.bfloat16
    f8 = ml_dtypes.float8_e4m3

    h_V = np.asarray(inputs["h_V"], dtype=np.float32)
    bid = np.asarray(inputs["batch_id"])
    W1 = np.asarray(inputs["W1"], dtype=np.float32)
    b1 = np.asarray(inputs["b1"], dtype=np.float32)
    W2 = np.asarray(inputs["W2"], dtype=np.float32)
    b2 = np.asarray(inputs["b2"], dtype=np.float32)

    h_bf = h_V.astype(bf16)
    bid_f = bid.astype(np.float32)
    w1t = np.ascontiguousarray(
        W1.T.reshape(KC, 128, H).transpose(1, 0, 2)).astype(bf16)
    w2t = np.ascontiguousarray(
        W2.T.reshape(KC, 128, H).transpose(1, 0, 2)).astype(bf16)
    b1c = np.ascontiguousarray(b1.reshape(KC, 128).T)
    b2c = np.ascontiguousarray(b2.reshape(KC, 128).T)
    iotar = np.ascontiguousarray(
        np.tile(np.arange(S, dtype=bf16), (128, 1)))
    iotac = np.arange(S, dtype=np.float32).reshape(S, 1)
    ones = np.ones((BLK, 1), dtype=bf16)
    ones8 = np.ones((BLK, 1), dtype=f8)
    ident = np.eye(128, dtype=bf16)

    in_maps = []
    for c in range(CORES):
        lo, hi = c * NPC, (c + 1) * NPC
        bid_c = bid_f[lo:hi]
        # bidq[p, j*BPI+b] = bid[j*L + p*BPI + b] (pass-1 per-block scalars)
        bidq = np.ascontiguousarray(
            bid_c.reshape(ITERS, BLK, BPI).transpose(1, 0, 2)
            .reshape(BLK, NBLK))
        # bidr[j*L + b*BLK + p] = bid[j*L + p*BPI + b] (pass-2 one-hot is
        # block-major so oht columns for block b are contiguous)
        bidr = np.ascontiguousarray(
            bid_c.reshape(ITERS, BLK, BPI).transpose(0, 2, 1)
            .reshape(1, NPC)).astype(bf16)
        unc = [j for j in range(ITERS) if not _is_cached(j)]
        in_maps.append({
            "hv": h_bf[lo:hi],
            "hv8": np.concatenate(
                [h_V[lo + j * L:lo + (j + 1) * L] for j in unc]).astype(f8),
            "bidq": bidq,
            "bidr": bidr,
            "w1t": w1t, "w2t": w2t, "b1c": b1c, "b2c": b2c,
            "iotar": iotar, "iotac": iotac, "ones": ones, "ones8": ones8,
            "ident": ident,
        })
    return in_maps


def _run(inputs, trace=False):
    global _cached
    if _cached is None:
        _cached = _build()
    nc = _cached
    in_maps = _prep_inputs(inputs)
    res = run_bass_kernel_spmd(nc, in_maps, core_ids=list(range(CORES)),
                               trace=trace)
    out = np.concatenate(
        [np.asarray(res.results[c]["out"]).astype(np.float32)
         for c in range(CORES)], axis=0)
    return out, res


def kernel(**inputs) -> np.ndarray:
    out, _ = _run(inputs, trace=False)
    return out


# revision 35
# speedup vs baseline: 1.1290x; 1.0044x over previous
"""Trainium2 Bass kernel for segment-mean -> gated-MLP -> gather-gate (nn_Context).

Math (reference):
    seg_sum[s] = sum_{n: bid[n]==s} h_V[n]          # [S, H]
    c_V = seg_sum / max(counts, 1)                  # [S, H]
    hdn = relu(c_V @ W1.T + b1)
    gate = sigmoid(hdn @ W2.T + b2)                 # [S, H]
    out[n] = h_V[n] * gate[bid[n]]                  # [N, H]

Strategy: shard nodes equally across 8 cores; h_V staged to DRAM in bf16
(tolerance 2e-2 >> bf16 rounding ~4e-3). Each core:
  pass 1: stream its h_V slice in 1 MiB tiles (8 blocks of 128 nodes per
          DMA); per block build a one-hot [128, S] on DVE and accumulate
          seg_sum [S, H] / counts [S, 1] on the PE into PSUM.  The tail
          CI iterations stay resident in SBUF (cache) for pass 2.
  AllReduce the packed [S, H+1] partials across the 8 cores.
  Tiny replicated MLP (bf16 weights) on PE + ACT -> gate [S, H] bf16.
  pass 2: per iteration build the transposed one-hot [S, L] (gpsimd
          partition-broadcast of the bid row + DVE is_equal), PE-matmul
          gathers gate rows, multiply with h_V (cached or re-read) and
          DMA out in bf16.  Host upcasts the result to f32.
"""

import numpy as np

import concourse.bass as bass
import concourse.mybir as mybir
import concourse.tile as tile
from concourse import bacc
from concourse.bass_utils import run_bass_kernel_spmd

N = 262144
H = 512
S = 64
CORES = 8
NPC = N // CORES       # 32768 nodes per core
BLK = 128              # nodes per matmul block (partition dim)
BPI = 8                # blocks per DMA iteration
L = BLK * BPI          # 1024 nodes per iteration
ITERS = NPC // L       # 32 iterations per pass
NBLK = NPC // BLK      # 256 matmul blocks
CI = 13                # iterations cached in SBUF between passes
CUT = ITERS - CI       # first cached iteration
KC = H // 128          # 4 column chunks of the hidden dim
F32 = mybir.dt.float32
BF16 = mybir.dt.bfloat16
F8 = mybir.dt.float8e4

EQ = mybir.AluOpType.is_equal
MULT = mybir.AluOpType.mult
AF = mybir.ActivationFunctionType

_cached = None  # (nc,) built once per process


def _is_cached(j):
    # Cached iterations are interleaved with streaming ones (odd j) so the
    # DMA engines never sit idle through a long cached-only tail: a streaming
    # iteration's input DMA overlaps its cached neighbour's compute.
    return j % 2 == 1 and j < 2 * CI


def _act_pairs(j):
    # Gating-multiply engine split per iteration (4 block-pairs each): pairs
    # below the returned count copy the gathered gate rows PSUM->SBUF (bf16,
    # ACT) so the DVE multiply runs in fast 16-bit mode; the rest multiply
    # straight from PSUM (f32) on DVE.  Cached iterations have no input DMA,
    # so ACT would otherwise become their pacer -> give one pair back to DVE.
    return 3 if _is_cached(j) else 4


def _build(use_collective=True, reps=1):
    nc = bacc.Bacc("TRN2", target_bir_lowering=False, debug=False,
                   num_devices=CORES if use_collective else None)

    hv_d = nc.dram_tensor("hv", [NPC, H], BF16, kind="ExternalInput")
    hv8_d = nc.dram_tensor("hv8", [CUT * L, H], F8, kind="ExternalInput")
    bidq_d = nc.dram_tensor("bidq", [BLK, NBLK], F32, kind="ExternalInput")
    bidr_d = nc.dram_tensor("bidr", [1, NPC], BF16, kind="ExternalInput")
    w1t_d = nc.dram_tensor("w1t", [128, KC, H], BF16, kind="ExternalInput")
    w2t_d = nc.dram_tensor("w2t", [128, KC, H], BF16, kind="ExternalInput")
    b1c_d = nc.dram_tensor("b1c", [128, KC], F32, kind="ExternalInput")
    b2c_d = nc.dram_tensor("b2c", [128, KC], F32, kind="ExternalInput")
    iotar_d = nc.dram_tensor("iotar", [128, S], BF16, kind="ExternalInput")
    iotac_d = nc.dram_tensor("iotac", [S, 1], F32, kind="ExternalInput")
    ones_d = nc.dram_tensor("ones", [BLK, 1], BF16, kind="ExternalInput")
    ones8_d = nc.dram_tensor("ones8", [BLK, 1], F8, kind="ExternalInput")
    ident_d = nc.dram_tensor("ident", [128, 128], BF16, kind="ExternalInput")
    out_d = nc.dram_tensor("out", [NPC, H], BF16, kind="ExternalOutput")

    # [i, p, b, h]: iteration i, block b, node = i*L + p*BPI + b
    hv_v = hv_d.ap().rearrange("(i p b) h -> i p b h", p=BLK, b=BPI)
    hv8_v = hv8_d.ap().rearrange("(i p b) h -> i p b h", p=BLK, b=BPI)
    out_v = out_d.ap().rearrange("(i p b) h -> i p b h", p=BLK, b=BPI)
    bidr_ap = bidr_d.ap()

    with tile.TileContext(nc) as tc:
        with (
            tc.tile_pool(name="const", bufs=1) as constp,
            tc.tile_pool(name="cache", bufs=1) as cachep,
            tc.tile_pool(name="hvp", bufs=4) as hvp,
            tc.tile_pool(name="hv8p", bufs=3) as hv8p,
            tc.tile_pool(name="rowp", bufs=3) as rowp,
            tc.tile_pool(name="ohp", bufs=4) as ohp,
            tc.tile_pool(name="bcp", bufs=3) as bcp,
            tc.tile_pool(name="ohtp", bufs=3) as ohtp,
            tc.tile_pool(name="gsbp", bufs=3) as gsbp,
            tc.tile_pool(name="outp", bufs=2) as outp,
            tc.tile_pool(name="smallp", bufs=1) as smallp,
            tc.tile_pool(name="dramp", bufs=1, space="DRAM") as dramp,
        ):
            # ---- constants ----
            bidq = constp.tile([BLK, NBLK], F32, tag="bidq")
            nc.scalar.dma_start(out=bidq[:], in_=bidq_d.ap())
            iotar = constp.tile([128, S], BF16, tag="iotar")
            nc.scalar.dma_start(out=iotar[:], in_=iotar_d.ap())
            iotac = constp.tile([S, 1], F32, tag="iotac")
            nc.scalar.dma_start(out=iotac[:], in_=iotac_d.ap())
            ones = constp.tile([BLK, 1], BF16, tag="ones")
            nc.scalar.dma_start(out=ones[:], in_=ones_d.ap())
            ones8 = constp.tile([BLK, 1], F8, tag="ones8")
            nc.scalar.dma_start(out=ones8[:], in_=ones8_d.ap())
            ident = constp.tile([128, 128], BF16, tag="ident")
            nc.scalar.dma_start(out=ident[:], in_=ident_d.ap())
            w1t = constp.tile([128, KC, H], BF16, tag="w1t")
            nc.scalar.dma_start(out=w1t[:], in_=w1t_d.ap())
            w2t = constp.tile([128, KC, H], BF16, tag="w2t")
            nc.scalar.dma_start(out=w2t[:], in_=w2t_d.ap())
            b1c = constp.tile([128, KC], F32, tag="b1c")
            nc.scalar.dma_start(out=b1c[:], in_=b1c_d.ap())
            b2c = constp.tile([128, KC], F32, tag="b2c")
            nc.scalar.dma_start(out=b2c[:], in_=b2c_d.ap())
            gate = constp.tile([S, H], BF16, tag="gate")
            hvc = cachep.tile([128, CI, BPI, H], BF16, tag="hvc")

            def body():
                _body(nc, tc, hv_v, hv8_v, out_v, bidr_ap, bidq, iotar, iotac,
                      ones, ones8, ident, w1t, w2t, b1c, b2c, gate, hvc, hvp,
                      hv8p, rowp, ohp, bcp, ohtp, gsbp, outp, smallp, dramp,
                      use_collective)

            if reps == 1:
                body()
            else:
                with tc.For_i(0, reps, 1):
                    body()

    nc.compile()
    return nc


def _body(nc, tc, hv_v, hv8_v, out_v, bidr_ap, bidq, iotar, iotac, ones,
          ones8, ident, w1t, w2t, b1c, b2c, gate, hvc, hvp, hv8p, rowp, ohp,
          bcp, ohtp, gsbp, outp, smallp, dramp, use_collective):
    with tc.tile_pool(name="psacc", bufs=1, space="PSUM") as psacc, \
         tc.tile_pool(name="psmlp", bufs=3, space="PSUM") as psmlp:
        # ---- pass 1: per-core seg_sum [S, H] and counts [S, 1] ----
        # Uncached iterations read an fp8 staging of h_V: only the segment
        # MEANS flow through it (error averages down over ~4k nodes/segment
        # and the gate is flat around 0.5), the gating multiply in pass 2
        # always uses bf16 data.
        pseg = psacc.tile([S, H], F32, tag="pseg")
        pcnt = psacc.tile([S, 1], F32, tag="pcnt")

        u = 0  # uncached-iteration ordinal (indexes the fp8 staging)
        for j in range(ITERS):
            f8 = not _is_cached(j)
            if not f8:
                hv_t = hvc[:, j // 2]
                nc.sync.dma_start(out=hv_t, in_=hv_v[j])
            else:
                hv_new = hv8p.tile([BLK, BPI, H], F8, tag="hv8")
                hv_t = hv_new[:]
                nc.sync.dma_start(out=hv_t, in_=hv8_v[u])
                u += 1
            for b in range(BPI):
                i = j * BPI + b
                oh = ohp.tile([BLK, S], F8 if f8 else BF16,
                              tag="oh8" if f8 else "oh")
                nc.vector.tensor_scalar(
                    out=oh[:], in0=iotar[:],
                    scalar1=bidq[:, i:i + 1], scalar2=None, op0=EQ)
                first = i == 0
                last = i == NBLK - 1
                nc.tensor.matmul(pseg[:], lhsT=oh[:], rhs=hv_t[:, b, :],
                                 start=first, stop=last)
                nc.tensor.matmul(pcnt[:], lhsT=oh[:],
                                 rhs=ones8[:] if f8 else ones[:],
                                 start=first, stop=last)

        # ---- AllReduce partial stats across the 8 cores ----
        pack = smallp.tile([S, H + 1], F32, tag="pack")
        nc.scalar.copy(pack[:, :H], pseg[:])
        nc.scalar.copy(pack[:, H:H + 1], pcnt[:])
        cc_in = dramp.tile([S, H + 1], F32, tag="ccin")
        cc_out = dramp.tile([S, H + 1], F32, tag="ccout")
        # Collective staging DMAs stay off the SP queue (pass-2 loads would
        # park behind the AllReduce wait) and use HWDGE queues (Act) rather
        # than Pool's SWDGE, whose fixed overheads sit on the serial mid
        # chain.
        nc.scalar.dma_start(out=cc_in[:], in_=pack[:])
        if use_collective:
            nc.gpsimd.collective_compute(
                "AllReduce",
                mybir.AluOpType.add,
                replica_groups=[list(range(CORES))],
                ins=[cc_in[:].opt()],
                outs=[cc_out[:].opt()],
            )
        else:  # single-core timing-model variant
            nc.gpsimd.dma_start(out=cc_out[:], in_=cc_in[:])
        packr = smallp.tile([S, H + 1], F32, tag="packr")
        nc.scalar.dma_start(out=packr[:], in_=cc_out[:])

        # ---- c_V = seg_sum / max(counts, 1) ----
        cnt = smallp.tile([S, 1], F32, tag="cnt")
        nc.vector.tensor_scalar_max(cnt[:], packr[:, H:H + 1], 1.0)
        rcp = smallp.tile([S, 1], F32, tag="rcp")
        nc.vector.reciprocal(rcp[:], cnt[:])
        cv = smallp.tile([S, H], BF16, tag="cv")
        nc.vector.tensor_scalar_mul(cv[:], packr[:, :H], rcp[:])

        # ---- transpose c_V -> ct [128, kc, S] (k on partitions) ----
        ct = smallp.tile([128, KC, S], BF16, tag="ct")
        for kc in range(KC):
            pt = psmlp.tile([128, S], BF16, tag="mlpt")
            nc.tensor.transpose(pt[:], in_=cv[:, kc * 128:(kc + 1) * 128],
                                identity=ident[:S, :S])
            nc.scalar.copy(ct[:, kc, :], pt[:])

        # ---- layer 1: h1_T[j, s] = relu(W1 @ c_V.T + b1) ----
        h1 = smallp.tile([128, KC, S], BF16, tag="h1")
        for jc in range(KC):
            ph = psmlp.tile([128, S], F32, tag="mlp")
            for kc in range(KC):
                nc.tensor.matmul(
                    ph[:], lhsT=w1t[:, kc, jc * 128:(jc + 1) * 128],
                    rhs=ct[:, kc, :], start=kc == 0, stop=kc == KC - 1)
            nc.scalar.activation(h1[:, jc, :], ph[:], AF.Relu,
                                 bias=b1c[:, jc:jc + 1])

        # ---- layer 2: g_T[m, s] = sigmoid(W2 @ h1 + b2) ----
        gt = smallp.tile([128, KC, S], BF16, tag="gt")
        for mc in range(KC):
            ph = psmlp.tile([128, S], F32, tag="mlp")
            for jc in range(KC):
                nc.tensor.matmul(
                    ph[:], lhsT=w2t[:, jc, mc * 128:(mc + 1) * 128],
                    rhs=h1[:, jc, :], start=jc == 0, stop=jc == KC - 1)
            nc.scalar.activation(gt[:, mc, :], ph[:], AF.Sigmoid,
                                 bias=b2c[:, mc:mc + 1])

        # ---- transpose back: gate [S, H] bf16 ----
        for mc in range(KC):
            pg = psmlp.tile([S, 128], BF16, tag="mlpt")
            nc.tensor.transpose(pg[:], in_=gt[:, mc, :], identity=ident[:])
            nc.scalar.copy(gate[:, mc * 128:(mc + 1) * 128], pg[:])

    # ---- pass 2: out = h_V * gate[bid] ----
    # Loads are software-pipelined LOOKAHEAD iterations in front of the
    # compute: SP's SEQ is in-order, so a load emitted after iteration j's
    # out-DMA cannot issue until j's multiplies finish.  Emitting the j+LA
    # loads before j's out-DMA keeps the DMA engines fed.  The refill for a
    # ring slot is emitted right after its previous consumer (WAR safety
    # requires lookahead < pool bufs).
    rowts = {}
    hvts = {}

    def row_load(jj):
        rowt = rowp.tile([1, L], BF16, tag="row")
        nc.sync.dma_start(out=rowt[:],
                          in_=bidr_ap[0:1, jj * L:(jj + 1) * L])
        rowts[jj] = rowt

    def hv_load(jj):
        if _is_cached(jj):
            return
        hv_new = hvp.tile([BLK, BPI, H], BF16, tag="hv1")
        nc.sync.dma_start(out=hv_new[:], in_=hv_v[jj])
        hvts[jj] = hv_new

    LA = 2  # lookahead; rowp has LA+1 bufs, hvp >= LA+2
    with tc.tile_pool(name="psg", bufs=4, space="PSUM") as psg:
        for jj in range(min(LA + 1, ITERS)):
            row_load(jj)
            hv_load(jj)
        for j in range(ITERS):
            rowt = rowts.pop(j)
            bct = bcp.tile([S, L], BF16, tag="bc")
            nc.gpsimd.partition_broadcast(bct[:], rowt[0:1, :], channels=S)
            if j + LA + 1 < ITERS:
                row_load(j + LA + 1)
            oht = ohtp.tile([S, L], BF16, tag="oht")
            nc.vector.tensor_scalar(
                out=oht[:], in0=bct[:],
                scalar1=iotac[:], scalar2=None, op0=EQ)
            if _is_cached(j):
                hv_t = hvc[:, j // 2]
            else:
                hv_t = hvts.pop(j)[:]
            if j + LA + 1 < ITERS:
                hv_load(j + LA + 1)
            ot = outp.tile([BLK, BPI, H], BF16, tag="ot")
            for q in range(BPI // 2):
                pgt = psg.tile([BLK, 2, H], F32, tag="pg2")
                for r in range(2):
                    b = 2 * q + r
                    nc.tensor.matmul(
                        pgt[:, r, :],
                        lhsT=oht[:, b * BLK:(b + 1) * BLK],
                        rhs=gate[:], start=True, stop=True)
                if q < _act_pairs(j):
                    gsb = gsbp.tile([BLK, 2, H], BF16, tag="gsb")
                    nc.scalar.copy(gsb[:], pgt[:])
                    nc.vector.tensor_tensor(
                        out=ot[:, 2 * q:2 * q + 2, :],
                        in0=hv_t[:, 2 * q:2 * q + 2, :], in1=gsb[:], op=MULT)
                else:
                    nc.vector.tensor_tensor(
                        out=ot[:, 2 * q:2 * q + 2, :],
                        in0=hv_t[:, 2 * q:2 * q + 2, :], in1=pgt[:], op=MULT)
            # out-DMA on SP: its wait (multiply done) matches the WAR waits
            # the SP prefetch loads already park on, so it adds no new
            # serialization; on Pool it blocked partition_broadcast, on Act
            # it stalled the PSUM->SBUF copy chain.
            nc.sync.dma_start(out=out_v[j], in_=ot[:])


def _prep_inputs(inputs):
    import ml_dtypes
    bf16 = ml_dtypes.bfloat16
    f8 = ml_dtypes.float8_e4m3

    h_V = np.asarray(inputs["h_V"], dtype=np.float32)
    bid = np.asarray(inputs["batch_id"])
    W1 = np.asarray(inputs["W1"], dtype=np.float32)
    b1 = np.asarray(inputs["b1"], dtype=np.float32)
    W2 = np.asarray(inputs["W2"], dtype=np.float32)
    b2 = np.asarray(inputs["b2"], dtype=np.float32)

    h_bf = h_V.astype(bf16)
    bid_f = bid.astype(np.float32)
    w1t = np.ascontiguousarray(
        W1.T.reshape(KC, 128, H).transpose(1, 0, 2)).astype(bf16)
    w2t = np.ascontiguousarray(
        W2.T.reshape(KC, 128, H).transpose(1, 0, 2)).astype(bf16)
    b1c = np.ascontiguousarray(b1.reshape(KC, 128).T)
    b2c = np.ascontiguousarray(b2.reshape(KC, 128).T)
    iotar = np.ascontiguousarray(
        np.tile(np.arange(S, dtype=bf16), (128, 1)))
    iotac = np.arange(S, dtype=np.float32).reshape(S, 1)
    ones = np.ones((BLK, 1), dtype=bf16)
    ones8 = np.ones((BLK, 1), dtype=f8)
    ident = np.eye(128, dtype=bf16)

    in_maps = []
    for c in range(CORES):
        lo, hi = c * NPC, (c + 1) * NPC
        bid_c = bid_f[lo:hi]
        # bidq[p, j*BPI+b] = bid[j*L + p*BPI + b] (pass-1 per-block scalars)
        bidq = np.ascontiguousarray(
            bid_c.reshape(ITERS, BLK, BPI).transpose(1, 0, 2)
            .reshape(BLK, NBLK))
        # bidr[j*L + b*BLK + p] = bid[j*L + p*BPI + b] (pass-2 one-hot is
        # block-major so oht columns for block b are contiguous)
        bidr = np.ascontiguousarray(
            bid_c.reshape(ITERS, BLK, BPI).transpose(0, 2, 1)
            .reshape(1, NPC)).astype(bf16)
        unc = [j for j in range(ITERS) if not _is_cached(j)]
        in_maps.append({
            "hv": h_bf[lo:hi],
            "hv8": np.concatenate(
                [h_V[lo + j * L:lo + (j + 1) * L] for j in unc]).astype(f8),
            "bidq": bidq,
            "bidr": bidr,
            "w1t": w1t, "w2t": w2t, "b1c": b1c, "b2c": b2c,
            "iotar": iotar, "iotac": iotac, "ones": ones, "ones8": ones8,
            "ident": ident,
        })
    return in_maps


def _run(inputs, trace=False):
    global _cached
    if _cached is None:
        _cached = _build()
    nc = _cached
    in_maps = _prep_inputs(inputs)
    res = run_bass_kernel_spmd(nc, in_maps, core_ids=list(range(CORES)),
                               trace=trace)
    out = np.concatenate(
        [np.asarray(res.results[c]["out"]).astype(np.float32)
         for c in range(CORES)], axis=0)
    return out, res


def kernel(**inputs) -> np.ndarray:
    out, _ = _run(inputs, trace=False)
    return out
